# revision 48
# baseline (speedup 1.0000x reference)
"""Trainium2 Bass kernel for the L1 tensor-product problem.

Math (per batch row b):
  out0e = [x0e*s, CG*(x1o.v)] @ W0e * NORM0E
  out0o = [x0o*s, CG*(x1e.v)] @ W0o * NORM0O
  out1e_c = [CG*x0o*v_c, CG*x1e_c*s, CGC*cross(x1o,v)_c] @ W1e * NORM1E
  out1o_c = [CG*x0e*v_c, CG*x1o_c*s, CGC*cross(x1e,v)_c] @ W1o * NORM1O

Kernel strategy (pure data parallel over batch, 8 cores), v2:
  * Everything bf16 on the wire and on the matmul path; PSUM accumulates
    fp32; output is written bf16 and upcast on the host (rel-err budget
    2e-2, bf16 gives ~3e-3).
  * Host packs x per core as [ntiles, 128, 10*T] so each partition's
    tile-load is one contiguous 10KB DMA descriptor (the fp32 baseline
    was descriptor-bound at ~2KB/descriptor, DMA 95% busy).
  * The per-row scalars (s, v_c) commute with the feature contraction:
    the x0o@W1e / x0e@W1o blocks are computed UNSCALED (2 matmuls each
    instead of 6 pre-scaled ones) and scaled on DVE afterwards.  This
    cuts matmuls per tile from 54 to 42.
  * The pre-scaled products are emitted as 2 wide DVE instructions
    (x_all * s; all 18 x1e/x1o * v_c chunks in one 4-D broadcast-AP op),
    the 0e/0o dot over components collapses 3 matmul K-chunks into 1 via
    2 paired DVE adds, and the v_c*g combine terms accumulate into PSUM
    through an identity matmul so no DVE add is needed.
  * Multiplier rows reach all 128 partitions via a stride-0 broadcast
    DMA read (GpSimd is unusable here: its tensor ops run far over the
    cost model and trip the chip's utilization throttle; the PE runs at
    a sustained 1.2GHz, so ones-matmul broadcasts are expensive too).
  * DVE is the bottleneck engine (~93% busy): the schedule software-
    pipelines stage B (combines + 1e/1o matmuls + store) one tile behind
    stage A and prefetches loads one tile ahead, so DVE never waits.
"""

import sys

sys.path.insert(0, "/opt/trn_rl_repo")

import numpy as np

import concourse.bass as bass
import concourse.bacc as bacc
import concourse.mybir as mybir
from concourse.bass_utils import run_bass_kernel_spmd
from concourse.tile import TileContext

N_CORES = 8
T = 512  # batch columns per tile

# irreps: 256x0e + 256x0o + 128x1e + 128x1o
CG = 1.0 / 3.0**0.5
CGC = 1.0 / 6.0**0.5
NORM0E = (1.0 / 384.0) ** 0.5
NORM0O = (1.0 / 384.0) ** 0.5
NORM1E = (3.0 / 512.0) ** 0.5
NORM1O = (3.0 / 512.0) ** 0.5

_BF16 = None


def _bf16():
    global _BF16
    if _BF16 is None:
        import ml_dtypes

        _BF16 = np.dtype(ml_dtypes.bfloat16)
    return _BF16


def _pack_weights(W0e, W0o, W1e, W1o):
    """Fold constants/signs; 22 lhsT chunks [128,128] side by side.

    Order: 0e (kc0m0,kc0m1,kc1m0,kc1m1,kc2m0,kc2m1), 0o (same 6),
    1e (g0,g1,h,k+,k-), 1o (g0,g1,h,k+,k-).
    """
    W0e = W0e.astype(np.float64) * NORM0E
    W0e[256:] *= CG
    W0o = W0o.astype(np.float64) * NORM0O
    W0o[256:] *= CG
    W1e = W1e.astype(np.float64) * NORM1E
    W1e[:384] *= CG
    W1e[384:] *= CGC
    W1o = W1o.astype(np.float64) * NORM1O
    W1o[:384] *= CG
    W1o[384:] *= CGC
    chunks = []
    for W in (W0e, W0o):  # [384, 256]
        for kc in range(3):
            for mc in range(2):
                chunks.append(W[kc * 128 : (kc + 1) * 128, mc * 128 : (mc + 1) * 128])
    for W in (W1e, W1o):  # [512, 128]
        chunks.append(W[0:128, :])      # g0
        chunks.append(W[128:256, :])    # g1
        chunks.append(W[256:384, :])    # h
        chunks.append(W[384:512, :])    # k+
        chunks.append(-W[384:512, :])   # k-
    chunks.append(np.eye(128, dtype=np.float64))  # 22: identity (combine accum)
    packed = np.concatenate(chunks, axis=1)
    return np.ascontiguousarray(packed.astype(_bf16()))


def _prep_shard(in1_s, in2_s):
    """in1 [Bs,1280] -> x [nt, 128, 10*T] bf16; in2 [Bs,4] -> s4 [nt,4,T].

    Chunk order: 0,1=x0e  2,3=x0o  4+c=x1e_c  7+c=x1o_c.
    """
    Bs = in1_s.shape[0]
    nt = Bs // T
    dt = _bf16()
    x = np.empty((nt, 128, 10, T), dt)
    x[:, :, 0:4] = in1_s[:, 0:512].reshape(nt, T, 4, 128).transpose(0, 3, 2, 1)
    x[:, :, 4:7] = in1_s[:, 512:896].reshape(nt, T, 128, 3).transpose(0, 2, 3, 1)
    x[:, :, 7:10] = in1_s[:, 896:1280].reshape(nt, T, 128, 3).transpose(0, 2, 3, 1)
    s4 = np.ascontiguousarray(in2_s.reshape(nt, T, 4).transpose(0, 2, 1).astype(dt))
    return np.ascontiguousarray(x.reshape(nt, 128, 10 * T)), s4


def _post_shard(y):
    """Device y [nt, 128, 10*T] bf16 -> [Bs, 1280] fp32 original layout."""
    nt = y.shape[0]
    y = np.asarray(y).reshape(nt, 128, 10, T).astype(np.float32)
    out = np.empty((nt, T, 1280), np.float32)
    out[:, :, 0:512] = y[:, :, 0:4].transpose(0, 3, 2, 1).reshape(nt, T, 512)
    out[:, :, 512:896] = y[:, :, 4:7].transpose(0, 3, 1, 2).reshape(nt, T, 384)
    out[:, :, 896:1280] = y[:, :, 7:10].transpose(0, 3, 1, 2).reshape(nt, T, 384)
    return out.reshape(nt * T, 1280)


def _build_program(Bs):
    assert Bs % T == 0, (Bs, T)
    nt = Bs // T
    bf = mybir.dt.bfloat16
    f32 = mybir.dt.float32

    nc = bacc.Bacc()
    x = nc.declare_dram_parameter("x", [nt, 128, 10 * T], bf, isOutput=False)
    s4 = nc.declare_dram_parameter("s4", [nt, 4, T], bf, isOutput=False)
    w = nc.declare_dram_parameter("w", [128, 23 * 128], bf, isOutput=False)
    y = nc.declare_dram_parameter("y", [nt, 128, 10 * T], bf, isOutput=True)

    with TileContext(nc) as tc:
        with (
            tc.tile_pool(name="wpool", bufs=1) as wpool,
            tc.tile_pool(name="xpool", bufs=3) as xpool,
            tc.tile_pool(name="mbpool", bufs=3) as mbpool,
            tc.tile_pool(name="pspool", bufs=3) as pspool,
            tc.tile_pool(name="pvpool", bufs=3) as pvpool,
            tc.tile_pool(name="cpool", bufs=2) as cpool,
            tc.tile_pool(name="ypool", bufs=2) as ypool,
            tc.tile_pool(name="psum", bufs=8, space="PSUM") as psum,
        ):
            wt = wpool.tile([128, 23 * 128], bf)
            nc.sync.dma_start(out=wt[:, :], in_=w[:, :])

            def W(i):
                return wt[:, i * 128 : (i + 1) * 128]

            def mm_into(p, contribs):
                # accumulate into an existing psum slice (one bank wide)
                n = len(contribs)
                for i, (wi, rhs) in enumerate(contribs):
                    nc.tensor.matmul(
                        p, W(wi), rhs, start=(i == 0), stop=(i == n - 1)
                    )

            def load(t):
                mbt = mbpool.tile([128, 4 * T], bf, tag="mb", name="mb_t")
                nc.sync.dma_start(
                    out=mbt[:, :].rearrange("p (c t) -> p c t", c=4),
                    in_=s4[t].unsqueeze(0).broadcast_to([128, 4, T]),
                )
                xt = xpool.tile([128, 10 * T], bf, tag="xt", name="x_t")
                # upper 6 chunks first: pvall (the big DVE op) needs only these
                nc.sync.dma_start(out=xt[:, 4 * T :], in_=x[t, :, 4 * T :])
                nc.sync.dma_start(out=xt[:, : 4 * T], in_=x[t, :, : 4 * T])
                return {"xt": xt, "mbt": mbt}

            def stage_g(st):
                # g = x0?' @ Wg (unscaled; only needs xt).  Emitted at the
                # head of each PE iteration so sgp is ready a full stage
                # before the t3 op that consumes it -- this breaks the
                # PE->Act->DVE chain that otherwise serializes the drain.
                xt = st["xt"]
                sgp = cpool.tile([128, 2 * T], bf, tag="sg", name="sg_t", bufs=4)
                for i, (wb, xg0) in enumerate(((12, 2), (17, 0))):
                    gp = psum.tile([128, T], f32, tag="psg", name="psg_t", bufs=2)
                    mm_into(
                        gp[:, :],
                        [
                            (wb + 0, xt[:, xg0 * T : (xg0 + 1) * T]),
                            (wb + 1, xt[:, (xg0 + 1) * T : (xg0 + 2) * T]),
                        ],
                    )
                    nc.scalar.copy(out=sgp[:, i * T : (i + 1) * T], in_=gp[:, :])
                st["sgp"] = sgp

            def stage_a(t, st):
                xt, mbt = st["xt"], st["mbt"]

                def mb(j, nch):
                    return (
                        mbt[:, j * T : (j + 1) * T]
                        .unsqueeze(1)
                        .broadcast_to([128, nch, T])
                    )

                # products: all 18 v-scaled chunks in ONE DVE op -- the
                # multiplier row advances by T per component c while the six
                # x1e/x1o chunks broadcast within each c
                pvall = pvpool.tile([128, 18 * T], bf, tag="pv", name="pv_t")
                nc.vector.tensor_mul(
                    pvall[:, :].rearrange("p (c k t) -> p c k t", c=3, k=6),
                    xt[:, 4 * T :]
                    .rearrange("p (k t) -> p k t", k=6)
                    .unsqueeze(1)
                    .broadcast_to([128, 3, 6, T]),
                    mbt[:, T:]
                    .rearrange("p (c t) -> p c t", c=3)
                    .unsqueeze(2)
                    .broadcast_to([128, 3, 6, T]),
                )

                def PV(c, ch):  # v_c-scaled chunk (ch is global 4..9)
                    o = c * 6 + (ch - 4)
                    return pvall[:, o * T : (o + 1) * T]

                def dpair(c):
                    # [128, 2, T]: {x1e_c*v_c (dot0o), x1o_c*v_c (dot0e)} --
                    # chunks c and 3+c within block c, a uniform 3T stride
                    o = c * 6 + c
                    return pvall[:, o * T : (o + 4) * T].rearrange(
                        "p (c t) -> p c t", c=4
                    )[:, 0::3, :]

                # dots for 0o|0e as a [128, 2T] pair (2 DVE ops, not 4)
                dta = cpool.tile([128, 2 * T], bf, tag="dta", name="dta_t", bufs=2)
                dotp = cpool.tile([128, 2 * T], bf, tag="dot", name="dot_t", bufs=2)
                dview = lambda ap: ap.rearrange("p (c t) -> p c t", c=2)
                nc.vector.tensor_add(dview(dta[:, :]), dpair(0), dpair(1))
                nc.vector.tensor_add(dview(dotp[:, :]), dview(dta[:, :]), dpair(2))
                dots = {0: dotp[:, T : 2 * T], 2: dotp[:, 0:T]}
                ps = pspool.tile([128, 10 * T], bf, tag="ps", name="ps_t")
                nc.vector.tensor_mul(
                    ps[:, :].rearrange("p (c t) -> p c t", c=10),
                    xt[:, :].rearrange("p (c t) -> p c t", c=10),
                    mb(0, 10),
                )

                def PS(ch):
                    return ps[:, ch * T : (ch + 1) * T]

                def XT(ch):
                    return xt[:, ch * T : (ch + 1) * T]

                yt = ypool.tile([128, 10 * T], bf, tag="yo", name="y_t")
                # 0e / 0o : both m-chunks in one [2T] psum, single Act copy
                # (the diag dot runs on DVE: a matmul here costs ~445ns at the
                # PE's sustained 1.2GHz clock, so un-collapsing loses)
                for base, wb, psa in ((0, 0, 0), (2, 6, 2)):
                    pp = psum.tile([128, 2 * T], f32, tag="ps0", name="ps0_t", bufs=2)
                    for m in range(2):
                        mm_into(
                            pp[:, m * T : (m + 1) * T],
                            [
                                (wb + 0 * 2 + m, PS(psa)),
                                (wb + 1 * 2 + m, PS(psa + 1)),
                                (wb + 2 * 2 + m, dots[base]),
                            ],
                        )
                    nc.scalar.copy(
                        out=yt[:, base * T : (base + 2) * T], in_=pp[:, :]
                    )
                nc.sync.dma_start(out=y[t, :, : 4 * T], in_=yt[:, : 4 * T])
                st.update({"ps": ps, "pvall": pvall, "yt": yt})

            def stage_b_dve(st):
                # t3[i,c] = v_c * g_i for both parities in one DVE op
                mbt, sgp = st["mbt"], st["sgp"]
                t3p = cpool.tile([128, 6 * T], bf, tag="t3", name="t3_t", bufs=4)
                nc.vector.tensor_mul(
                    t3p[:, :].rearrange("p (i c t) -> p i c t", i=2, c=3),
                    mbt[:, T:]
                    .rearrange("p (c t) -> p c t", c=3)
                    .unsqueeze(1)
                    .broadcast_to([128, 2, 3, T]),
                    sgp[:, :]
                    .rearrange("p (i t) -> p i t", i=2)
                    .unsqueeze(2)
                    .broadcast_to([128, 2, 3, T]),
                )
                st["t3p"] = t3p

            def stage_b_pe(t, st):
                ps, pvall, yt, t3p = st["ps"], st["pvall"], st["yt"], st["t3p"]

                def PS(ch):
                    return ps[:, ch * T : (ch + 1) * T]

                def PV(c, ch):
                    o = c * 6 + (ch - 4)
                    return pvall[:, o * T : (o + 1) * T]

                for i, (wb, hb, cb, ob) in enumerate(
                    ((12, 4, 7, 4), (17, 7, 4, 7))
                ):
                    t3 = t3p[:, i * 3 * T : (i + 1) * 3 * T]

                    def contribs(c):
                        a, b = (c + 1) % 3, (c + 2) % 3
                        return [
                            (wb + 3, PV(b, cb + a)),      # k+: x1op_a * v_b
                            (wb + 4, PV(a, cb + b)),      # k-: x1op_b * v_a
                            (22, t3[:, c * T : (c + 1) * T]),  # += v_c * g
                            (wb + 2, PS(hb + c)),         # h: x1par_c * s (last:
                        ]                                 # ps lands latest)

                    # components 0,1 share a [2T] psum + one copy; c=2 alone
                    pp = psum.tile([128, 2 * T], f32, tag="ps1", name="ps1_t", bufs=1)
                    for c in range(2):
                        mm_into(pp[:, c * T : (c + 1) * T], contribs(c))
                    pc2 = psum.tile([128, T], f32, tag="psg", name="ps1c_t", bufs=2)
                    mm_into(pc2[:, :], contribs(2))
                    nc.scalar.copy(out=yt[:, ob * T : (ob + 2) * T], in_=pp[:, :])
                    nc.scalar.copy(
                        out=yt[:, (ob + 2) * T : (ob + 3) * T], in_=pc2[:, :]
                    )
                    # stream this parity's output while the other computes
                    nc.sync.dma_start(
                        out=y[t, :, ob * T : (ob + 3) * T],
                        in_=yt[:, ob * T : (ob + 3) * T],
                    )

            # software pipeline: loads prefetched one tile ahead, stage B
            # (combines + 1e/1o matmuls + store) one tile behind stage A
            states = {0: load(0)}
            for t in range(nt):
                if t + 1 < nt:
                    states[t + 1] = load(t + 1)
                stage_g(states[t])
                if t >= 1:
                    stage_b_dve(states[t - 1])
                    stage_b_pe(t - 1, states[t - 1])
                stage_a(t, states[t])
                if t >= 1:
                    del states[t - 1]
            stage_b_dve(states[nt - 1])
            stage_b_pe(nt - 1, states[nt - 1])
    nc.finalize()
    return nc


_PROG_CACHE = {}


def _get_program(Bs):
    if Bs not in _PROG_CACHE:
        _PROG_CACHE[Bs] = _build_program(Bs)
    return _PROG_CACHE[Bs]


def run(inputs, trace=False, **kw):
    in1 = np.asarray(inputs["in1"], np.float32)
    in2 = np.asarray(inputs["in2"], np.float32)
    B = in1.shape[0]
    assert B % (N_CORES * T) == 0, B
    Bs = B // N_CORES

    wpk = _pack_weights(
        np.asarray(inputs["W0e"], np.float32),
        np.asarray(inputs["W0o"], np.float32),
        np.asarray(inputs["W1e"], np.float32),
        np.asarray(inputs["W1o"], np.float32),
    )

    in_maps = []
    for i in range(N_CORES):
        ssl = slice(i * Bs, (i + 1) * Bs)
        xs, s4s = _prep_shard(in1[ssl], in2[ssl])
        in_maps.append({"x": xs, "s4": s4s, "w": wpk})

    nc = _get_program(Bs)
    res = run_bass_kernel_spmd(nc, in_maps, list(range(N_CORES)), trace=trace, **kw)

    out = np.empty((B, 1280), np.float32)
    for i in range(N_CORES):
        out[i * Bs : (i + 1) * Bs] = _post_shard(res.results[i]["y"])
    return out, res


def kernel(**inputs):
    out, _ = run(inputs, trace=False)
    return out


# revision 49
# speedup vs baseline: 1.1854x; 1.1854x over previous
"""Trainium2 Bass kernel for the L1 tensor-product problem.

Math (per batch row b):
  out0e = [x0e*s, CG*(x1o.v)] @ W0e * NORM0E
  out0o = [x0o*s, CG*(x1e.v)] @ W0o * NORM0O
  out1e_c = [CG*x0o*v_c, CG*x1e_c*s, CGC*cross(x1o,v)_c] @ W1e * NORM1E
  out1o_c = [CG*x0e*v_c, CG*x1o_c*s, CGC*cross(x1e,v)_c] @ W1o * NORM1O

Kernel strategy (pure data parallel over batch, 8 cores), v2:
  * Everything bf16 on the wire and on the matmul path; PSUM accumulates
    fp32; output is written bf16 and upcast on the host (rel-err budget
    2e-2, bf16 gives ~3e-3).
  * Host packs x per core as [ntiles, 128, 10*T] so each partition's
    tile-load is one contiguous 10KB DMA descriptor (the fp32 baseline
    was descriptor-bound at ~2KB/descriptor, DMA 95% busy).
  * The per-row scalars (s, v_c) commute with the feature contraction:
    the x0o@W1e / x0e@W1o blocks are computed UNSCALED (2 matmuls each
    instead of 6 pre-scaled ones) and scaled on DVE afterwards.  This
    cuts matmuls per tile from 54 to 42.
  * The pre-scaled products are emitted as 2 wide DVE instructions
    (x_all * s; all 18 x1e/x1o * v_c chunks in one 4-D broadcast-AP op),
    the 0e/0o dot over components collapses 3 matmul K-chunks into 1 via
    2 paired DVE adds, and the v_c*g combine terms accumulate into PSUM
    through an identity matmul so no DVE add is needed.
  * Multiplier rows reach all 128 partitions via a stride-0 broadcast
    DMA read (GpSimd is unusable here: its tensor ops run far over the
    cost model and trip the chip's utilization throttle; the PE runs at
    a sustained 1.2GHz, so ones-matmul broadcasts are expensive too).
  * DVE is the bottleneck engine (~93% busy): the schedule software-
    pipelines stage B (combines + 1e/1o matmuls + store) one tile behind
    stage A and prefetches loads one tile ahead, so DVE never waits.
"""

import sys

sys.path.insert(0, "/opt/trn_rl_repo")

import numpy as np

import concourse.bass as bass
import concourse.bacc as bacc
import concourse.mybir as mybir
from concourse.bass_utils import run_bass_kernel_spmd
from concourse.tile import TileContext

N_CORES = 8
T = 512  # batch columns per tile

# irreps: 256x0e + 256x0o + 128x1e + 128x1o
CG = 1.0 / 3.0**0.5
CGC = 1.0 / 6.0**0.5
NORM0E = (1.0 / 384.0) ** 0.5
NORM0O = (1.0 / 384.0) ** 0.5
NORM1E = (3.0 / 512.0) ** 0.5
NORM1O = (3.0 / 512.0) ** 0.5

_BF16 = None


def _bf16():
    global _BF16
    if _BF16 is None:
        import ml_dtypes

        _BF16 = np.dtype(ml_dtypes.bfloat16)
    return _BF16


def _pack_weights(W0e, W0o, W1e, W1o):
    """Fold constants/signs; 22 lhsT chunks [128,128] side by side.

    Order: 0e (kc0m0,kc0m1,kc1m0,kc1m1,kc2m0,kc2m1), 0o (same 6),
    1e (g0,g1,h,k+,k-), 1o (g0,g1,h,k+,k-).
    """
    W0e = W0e.astype(np.float64) * NORM0E
    W0e[256:] *= CG
    W0o = W0o.astype(np.float64) * NORM0O
    W0o[256:] *= CG
    W1e = W1e.astype(np.float64) * NORM1E
    W1e[:384] *= CG
    W1e[384:] *= CGC
    W1o = W1o.astype(np.float64) * NORM1O
    W1o[:384] *= CG
    W1o[384:] *= CGC
    chunks = []
    for W in (W0e, W0o):  # [384, 256]
        for kc in range(3):
            for mc in range(2):
                chunks.append(W[kc * 128 : (kc + 1) * 128, mc * 128 : (mc + 1) * 128])
    for W in (W1e, W1o):  # [512, 128]
        chunks.append(W[0:128, :])      # g0
        chunks.append(W[128:256, :])    # g1
        chunks.append(W[256:384, :])    # h
        chunks.append(W[384:512, :])    # k+
        chunks.append(-W[384:512, :])   # k-
    chunks.append(np.eye(128, dtype=np.float64))  # 22: identity (combine accum)
    packed = np.concatenate(chunks, axis=1)
    return np.ascontiguousarray(packed.astype(_bf16()))


def _prep_shard(in1_s, in2_s):
    """in1 [Bs,1280] -> x [nt, 128, 10*T] bf16; in2 [Bs,4] -> s4 [nt,4,T].

    Chunk order: 0,1=x0e  2,3=x0o  4+c=x1e_c  7+c=x1o_c.
    """
    Bs = in1_s.shape[0]
    nt = Bs // T
    dt = _bf16()
    x = np.empty((nt, 128, 10, T), dt)
    x[:, :, 0:4] = in1_s[:, 0:512].reshape(nt, T, 4, 128).transpose(0, 3, 2, 1)
    x[:, :, 4:7] = in1_s[:, 512:896].reshape(nt, T, 128, 3).transpose(0, 2, 3, 1)
    x[:, :, 7:10] = in1_s[:, 896:1280].reshape(nt, T, 128, 3).transpose(0, 2, 3, 1)
    s4 = np.ascontiguousarray(in2_s.reshape(nt, T, 4).transpose(0, 2, 1).astype(dt))
    return np.ascontiguousarray(x.reshape(nt, 128, 10 * T)), s4


def _post_shard(y):
    """Device y [nt, 128, 10*T] bf16 -> [Bs, 1280] fp32 original layout."""
    nt = y.shape[0]
    y = np.asarray(y).reshape(nt, 128, 10, T).astype(np.float32)
    out = np.empty((nt, T, 1280), np.float32)
    out[:, :, 0:512] = y[:, :, 0:4].transpose(0, 3, 2, 1).reshape(nt, T, 512)
    out[:, :, 512:896] = y[:, :, 4:7].transpose(0, 3, 1, 2).reshape(nt, T, 384)
    out[:, :, 896:1280] = y[:, :, 7:10].transpose(0, 3, 1, 2).reshape(nt, T, 384)
    return out.reshape(nt * T, 1280)


def _build_program(Bs):
    assert Bs % T == 0, (Bs, T)
    nt = Bs // T
    bf = mybir.dt.bfloat16
    f32 = mybir.dt.float32

    nc = bacc.Bacc()
    x = nc.declare_dram_parameter("x", [nt, 128, 10 * T], bf, isOutput=False)
    s4 = nc.declare_dram_parameter("s4", [nt, 4, T], bf, isOutput=False)
    w = nc.declare_dram_parameter("w", [128, 23 * 128], bf, isOutput=False)
    y = nc.declare_dram_parameter("y", [nt, 128, 10 * T], bf, isOutput=True)

    with TileContext(nc) as tc:
        with (
            tc.tile_pool(name="wpool", bufs=1) as wpool,
            tc.tile_pool(name="xpool", bufs=3) as xpool,
            tc.tile_pool(name="mbpool", bufs=3) as mbpool,
            tc.tile_pool(name="pspool", bufs=3) as pspool,
            tc.tile_pool(name="pvpool", bufs=3) as pvpool,
            tc.tile_pool(name="cpool", bufs=2) as cpool,
            tc.tile_pool(name="ypool", bufs=2) as ypool,
            tc.tile_pool(name="psum", bufs=8, space="PSUM") as psum,
        ):
            wt = wpool.tile([128, 23 * 128], bf)
            nc.sync.dma_start(out=wt[:, :], in_=w[:, :])

            def W(i):
                return wt[:, i * 128 : (i + 1) * 128]

            def mm_into(p, contribs):
                # accumulate into an existing psum slice (one bank wide)
                n = len(contribs)
                for i, (wi, rhs) in enumerate(contribs):
                    nc.tensor.matmul(
                        p, W(wi), rhs, start=(i == 0), stop=(i == n - 1)
                    )

            def load(t):
                mbt = mbpool.tile([128, 4 * T], bf, tag="mb", name="mb_t")
                nc.sync.dma_start(
                    out=mbt[:, :].rearrange("p (c t) -> p c t", c=4),
                    in_=s4[t].unsqueeze(0).broadcast_to([128, 4, T]),
                )
                xt = xpool.tile([128, 10 * T], bf, tag="xt", name="x_t")
                # upper 6 chunks first: pvall (the big DVE op) needs only these
                nc.sync.dma_start(out=xt[:, 4 * T :], in_=x[t, :, 4 * T :])
                nc.sync.dma_start(out=xt[:, : 4 * T], in_=x[t, :, : 4 * T])
                return {"xt": xt, "mbt": mbt}

            def stage_g(st):
                # g = x0?' @ Wg (unscaled; only needs xt).  Emitted at the
                # head of each PE iteration so sgp is ready a full stage
                # before the t3 op that consumes it -- this breaks the
                # PE->Act->DVE chain that otherwise serializes the drain.
                xt = st["xt"]
                sgp = cpool.tile([128, 2 * T], bf, tag="sg", name="sg_t", bufs=4)
                for i, (wb, xg0) in enumerate(((12, 2), (17, 0))):
                    gp = psum.tile([128, T], f32, tag="psg", name="psg_t", bufs=2)
                    mm_into(
                        gp[:, :],
                        [
                            (wb + 0, xt[:, xg0 * T : (xg0 + 1) * T]),
                            (wb + 1, xt[:, (xg0 + 1) * T : (xg0 + 2) * T]),
                        ],
                    )
                    nc.scalar.copy(out=sgp[:, i * T : (i + 1) * T], in_=gp[:, :])
                st["sgp"] = sgp

            def stage_a(t, st):
                xt, mbt = st["xt"], st["mbt"]

                def mb(j, nch):
                    return (
                        mbt[:, j * T : (j + 1) * T]
                        .unsqueeze(1)
                        .broadcast_to([128, nch, T])
                    )

                # products: all 18 v-scaled chunks in ONE DVE op -- the
                # multiplier row advances by T per component c while the six
                # x1e/x1o chunks broadcast within each c
                pvall = pvpool.tile([128, 18 * T], bf, tag="pv", name="pv_t")
                nc.vector.tensor_mul(
                    pvall[:, :].rearrange("p (c k t) -> p c k t", c=3, k=6),
                    xt[:, 4 * T :]
                    .rearrange("p (k t) -> p k t", k=6)
                    .unsqueeze(1)
                    .broadcast_to([128, 3, 6, T]),
                    mbt[:, T:]
                    .rearrange("p (c t) -> p c t", c=3)
                    .unsqueeze(2)
                    .broadcast_to([128, 3, 6, T]),
                )

                def PV(c, ch):  # v_c-scaled chunk (ch is global 4..9)
                    o = c * 6 + (ch - 4)
                    return pvall[:, o * T : (o + 1) * T]

                def dpair(c):
                    # [128, 2, T]: {x1e_c*v_c (dot0o), x1o_c*v_c (dot0e)} --
                    # chunks c and 3+c within block c, a uniform 3T stride
                    o = c * 6 + c
                    return pvall[:, o * T : (o + 4) * T].rearrange(
                        "p (c t) -> p c t", c=4
                    )[:, 0::3, :]

                # dots for 0o|0e as a [128, 2T] pair (2 DVE ops, not 4)
                dta = cpool.tile([128, 2 * T], bf, tag="dta", name="dta_t", bufs=2)
                dotp = cpool.tile([128, 2 * T], bf, tag="dot", name="dot_t", bufs=2)
                dview = lambda ap: ap.rearrange("p (c t) -> p c t", c=2)
                nc.vector.tensor_add(dview(dta[:, :]), dpair(0), dpair(1))
                nc.vector.tensor_add(dview(dotp[:, :]), dview(dta[:, :]), dpair(2))
                dots = {0: dotp[:, T : 2 * T], 2: dotp[:, 0:T]}
                ps = pspool.tile([128, 10 * T], bf, tag="ps", name="ps_t")
                nc.vector.tensor_mul(
                    ps[:, :].rearrange("p (c t) -> p c t", c=10),
                    xt[:, :].rearrange("p (c t) -> p c t", c=10),
                    mb(0, 10),
                )

                def PS(ch):
                    return ps[:, ch * T : (ch + 1) * T]

                def XT(ch):
                    return xt[:, ch * T : (ch + 1) * T]

                yt = ypool.tile([128, 10 * T], bf, tag="yo", name="y_t")
                # 0e / 0o : both m-chunks in one [2T] psum, single Act copy
                # (the diag dot runs on DVE: a matmul here costs ~445ns at the
                # PE's sustained 1.2GHz clock, so un-collapsing loses)
                for base, wb, psa in ((0, 0, 0), (2, 6, 2)):
                    pp = psum.tile([128, 2 * T], f32, tag="ps0", name="ps0_t", bufs=2)
                    for m in range(2):
                        mm_into(
                            pp[:, m * T : (m + 1) * T],
                            [
                                (wb + 0 * 2 + m, PS(psa)),
                                (wb + 1 * 2 + m, PS(psa + 1)),
                                (wb + 2 * 2 + m, dots[base]),
                            ],
                        )
                    nc.scalar.copy(
                        out=yt[:, base * T : (base + 2) * T], in_=pp[:, :]
                    )
                    # stream this parity pair's output immediately
                    nc.sync.dma_start(
                        out=y[t, :, base * T : (base + 2) * T],
                        in_=yt[:, base * T : (base + 2) * T],
                    )
                st.update({"ps": ps, "pvall": pvall, "yt": yt})

            def stage_b_dve(st):
                # t3[i,c] = v_c * g_i for both parities in one DVE op
                mbt, sgp = st["mbt"], st["sgp"]
                t3p = cpool.tile([128, 6 * T], bf, tag="t3", name="t3_t", bufs=4)
                nc.vector.tensor_mul(
                    t3p[:, :].rearrange("p (i c t) -> p i c t", i=2, c=3),
                    mbt[:, T:]
                    .rearrange("p (c t) -> p c t", c=3)
                    .unsqueeze(1)
                    .broadcast_to([128, 2, 3, T]),
                    sgp[:, :]
                    .rearrange("p (i t) -> p i t", i=2)
                    .unsqueeze(2)
                    .broadcast_to([128, 2, 3, T]),
                )
                st["t3p"] = t3p

            def stage_b_pe(t, st):
                ps, pvall, yt, t3p = st["ps"], st["pvall"], st["yt"], st["t3p"]

                def PS(ch):
                    return ps[:, ch * T : (ch + 1) * T]

                def PV(c, ch):
                    o = c * 6 + (ch - 4)
                    return pvall[:, o * T : (o + 1) * T]

                for i, (wb, hb, cb, ob) in enumerate(
                    ((12, 4, 7, 4), (17, 7, 4, 7))
                ):
                    t3 = t3p[:, i * 3 * T : (i + 1) * 3 * T]

                    def contribs(c):
                        a, b = (c + 1) % 3, (c + 2) % 3
                        return [
                            (wb + 3, PV(b, cb + a)),      # k+: x1op_a * v_b
                            (wb + 4, PV(a, cb + b)),      # k-: x1op_b * v_a
                            (22, t3[:, c * T : (c + 1) * T]),  # += v_c * g
                            (wb + 2, PS(hb + c)),         # h: x1par_c * s (last:
                        ]                                 # ps lands latest)

                    # components 0,1 share a [2T] psum + one copy; c=2 alone
                    pp = psum.tile([128, 2 * T], f32, tag="ps1", name="ps1_t", bufs=1)
                    for c in range(2):
                        mm_into(pp[:, c * T : (c + 1) * T], contribs(c))
                    pc2 = psum.tile([128, T], f32, tag="psg", name="ps1c_t", bufs=2)
                    mm_into(pc2[:, :], contribs(2))
                    nc.scalar.copy(out=yt[:, ob * T : (ob + 2) * T], in_=pp[:, :])
                    nc.scalar.copy(
                        out=yt[:, (ob + 2) * T : (ob + 3) * T], in_=pc2[:, :]
                    )
                    # stream this parity's output while the other computes
                    nc.sync.dma_start(
                        out=y[t, :, ob * T : (ob + 3) * T],
                        in_=yt[:, ob * T : (ob + 3) * T],
                    )

            # software pipeline: loads prefetched one tile ahead, stage B
            # (combines + 1e/1o matmuls + store) one tile behind stage A
            states = {0: load(0)}
            for t in range(nt):
                if t + 1 < nt:
                    states[t + 1] = load(t + 1)
                stage_g(states[t])
                if t >= 1:
                    stage_b_dve(states[t - 1])
                    stage_b_pe(t - 1, states[t - 1])
                stage_a(t, states[t])
                if t >= 1:
                    del states[t - 1]
            stage_b_dve(states[nt - 1])
            stage_b_pe(nt - 1, states[nt - 1])
    nc.finalize()
    return nc


_PROG_CACHE = {}


def _get_program(Bs):
    if Bs not in _PROG_CACHE:
        _PROG_CACHE[Bs] = _build_program(Bs)
    return _PROG_CACHE[Bs]


def run(inputs, trace=False, **kw):
    in1 = np.asarray(inputs["in1"], np.float32)
    in2 = np.asarray(inputs["in2"], np.float32)
    B = in1.shape[0]
    assert B % (N_CORES * T) == 0, B
    Bs = B // N_CORES

    wpk = _pack_weights(
        np.asarray(inputs["W0e"], np.float32),
        np.asarray(inputs["W0o"], np.float32),
        np.asarray(inputs["W1e"], np.float32),
        np.asarray(inputs["W1o"], np.float32),
    )

    in_maps = []
    for i in range(N_CORES):
        ssl = slice(i * Bs, (i + 1) * Bs)
        xs, s4s = _prep_shard(in1[ssl], in2[ssl])
        in_maps.append({"x": xs, "s4": s4s, "w": wpk})

    nc = _get_program(Bs)
    res = run_bass_kernel_spmd(nc, in_maps, list(range(N_CORES)), trace=trace, **kw)

    out = np.empty((B, 1280), np.float32)
    for i in range(N_CORES):
        out[i * Bs : (i + 1) * Bs] = _post_shard(res.results[i]["y"])
    return out, res


def kernel(**inputs):
    out, _ = run(inputs, trace=False)
    return out


# revision 50
# speedup vs baseline: 1.1924x; 1.0059x over previous
"""Trainium2 Bass kernel for the L1 tensor-product problem.

Math (per batch row b):
  out0e = [x0e*s, CG*(x1o.v)] @ W0e * NORM0E
  out0o = [x0o*s, CG*(x1e.v)] @ W0o * NORM0O
  out1e_c = [CG*x0o*v_c, CG*x1e_c*s, CGC*cross(x1o,v)_c] @ W1e * NORM1E
  out1o_c = [CG*x0e*v_c, CG*x1o_c*s, CGC*cross(x1e,v)_c] @ W1o * NORM1O

Kernel strategy (pure data parallel over batch, 8 cores), v2:
  * Everything bf16 on the wire and on the matmul path; PSUM accumulates
    fp32; output is written bf16 and upcast on the host (rel-err budget
    2e-2, bf16 gives ~3e-3).
  * Host packs x per core as [ntiles, 128, 10*T] so each partition's
    tile-load is one contiguous 10KB DMA descriptor (the fp32 baseline
    was descriptor-bound at ~2KB/descriptor, DMA 95% busy).
  * The per-row scalars (s, v_c) commute with the feature contraction:
    the x0o@W1e / x0e@W1o blocks are computed UNSCALED (2 matmuls each
    instead of 6 pre-scaled ones) and scaled on DVE afterwards.  This
    cuts matmuls per tile from 54 to 42.
  * The pre-scaled products are emitted as 2 wide DVE instructions
    (x_all * s; all 18 x1e/x1o * v_c chunks in one 4-D broadcast-AP op),
    the 0e/0o dot over components collapses 3 matmul K-chunks into 1 via
    2 paired DVE adds, and the v_c*g combine terms accumulate into PSUM
    through an identity matmul so no DVE add is needed.
  * Multiplier rows reach all 128 partitions via a stride-0 broadcast
    DMA read (GpSimd is unusable here: its tensor ops run far over the
    cost model and trip the chip's utilization throttle; the PE runs at
    a sustained 1.2GHz, so ones-matmul broadcasts are expensive too).
  * DVE is the bottleneck engine (~93% busy): the schedule software-
    pipelines stage B (combines + 1e/1o matmuls + store) one tile behind
    stage A and prefetches loads one tile ahead, so DVE never waits.
"""

import sys

sys.path.insert(0, "/opt/trn_rl_repo")

import numpy as np

import concourse.bass as bass
import concourse.bacc as bacc
import concourse.mybir as mybir
from concourse.bass_utils import run_bass_kernel_spmd
from concourse.tile import TileContext

N_CORES = 8
T = 512  # batch columns per tile

# irreps: 256x0e + 256x0o + 128x1e + 128x1o
CG = 1.0 / 3.0**0.5
CGC = 1.0 / 6.0**0.5
NORM0E = (1.0 / 384.0) ** 0.5
NORM0O = (1.0 / 384.0) ** 0.5
NORM1E = (3.0 / 512.0) ** 0.5
NORM1O = (3.0 / 512.0) ** 0.5

_BF16 = None


def _bf16():
    global _BF16
    if _BF16 is None:
        import ml_dtypes

        _BF16 = np.dtype(ml_dtypes.bfloat16)
    return _BF16


def _pack_weights(W0e, W0o, W1e, W1o):
    """Fold constants/signs; 22 lhsT chunks [128,128] side by side.

    Order: 0e (kc0m0,kc0m1,kc1m0,kc1m1,kc2m0,kc2m1), 0o (same 6),
    1e (g0,g1,h,k+,k-), 1o (g0,g1,h,k+,k-).
    """
    W0e = W0e.astype(np.float64) * NORM0E
    W0e[256:] *= CG
    W0o = W0o.astype(np.float64) * NORM0O
    W0o[256:] *= CG
    W1e = W1e.astype(np.float64) * NORM1E
    W1e[:384] *= CG
    W1e[384:] *= CGC
    W1o = W1o.astype(np.float64) * NORM1O
    W1o[:384] *= CG
    W1o[384:] *= CGC
    chunks = []
    for W in (W0e, W0o):  # [384, 256]
        for kc in range(3):
            for mc in range(2):
                chunks.append(W[kc * 128 : (kc + 1) * 128, mc * 128 : (mc + 1) * 128])
    for W in (W1e, W1o):  # [512, 128]
        chunks.append(W[0:128, :])      # g0
        chunks.append(W[128:256, :])    # g1
        chunks.append(W[256:384, :])    # h
        chunks.append(W[384:512, :])    # k+
        chunks.append(-W[384:512, :])   # k-
    chunks.append(np.eye(128, dtype=np.float64))  # 22: identity (combine accum)
    packed = np.concatenate(chunks, axis=1)
    return np.ascontiguousarray(packed.astype(_bf16()))


def _prep_shard(in1_s, in2_s):
    """in1 [Bs,1280] -> x [nt, 128, 10*T] bf16; in2 [Bs,4] -> s4 [nt,4,T].

    Chunk order: 0,1=x0e  2,3=x0o  4+c=x1e_c  7+c=x1o_c.
    """
    Bs = in1_s.shape[0]
    nt = Bs // T
    dt = _bf16()
    x = np.empty((nt, 128, 10, T), dt)
    x[:, :, 0:4] = in1_s[:, 0:512].reshape(nt, T, 4, 128).transpose(0, 3, 2, 1)
    x[:, :, 4:7] = in1_s[:, 512:896].reshape(nt, T, 128, 3).transpose(0, 2, 3, 1)
    x[:, :, 7:10] = in1_s[:, 896:1280].reshape(nt, T, 128, 3).transpose(0, 2, 3, 1)
    s4 = np.ascontiguousarray(in2_s.reshape(nt, T, 4).transpose(0, 2, 1).astype(dt))
    return np.ascontiguousarray(x.reshape(nt, 128, 10 * T)), s4


def _post_shard(y):
    """Device y [nt, 128, 10*T] bf16 -> [Bs, 1280] fp32 original layout."""
    nt = y.shape[0]
    y = np.asarray(y).reshape(nt, 128, 10, T).astype(np.float32)
    out = np.empty((nt, T, 1280), np.float32)
    out[:, :, 0:512] = y[:, :, 0:4].transpose(0, 3, 2, 1).reshape(nt, T, 512)
    out[:, :, 512:896] = y[:, :, 4:7].transpose(0, 3, 1, 2).reshape(nt, T, 384)
    out[:, :, 896:1280] = y[:, :, 7:10].transpose(0, 3, 1, 2).reshape(nt, T, 384)
    return out.reshape(nt * T, 1280)


def _build_program(Bs):
    assert Bs % T == 0, (Bs, T)
    nt = Bs // T
    bf = mybir.dt.bfloat16
    f32 = mybir.dt.float32

    nc = bacc.Bacc()
    x = nc.declare_dram_parameter("x", [nt, 128, 10 * T], bf, isOutput=False)
    s4 = nc.declare_dram_parameter("s4", [nt, 4, T], bf, isOutput=False)
    w = nc.declare_dram_parameter("w", [128, 23 * 128], bf, isOutput=False)
    y = nc.declare_dram_parameter("y", [nt, 128, 10 * T], bf, isOutput=True)

    with TileContext(nc) as tc:
        with (
            tc.tile_pool(name="wpool", bufs=1) as wpool,
            tc.tile_pool(name="xpool", bufs=3) as xpool,
            tc.tile_pool(name="mbpool", bufs=3) as mbpool,
            tc.tile_pool(name="pspool", bufs=3) as pspool,
            tc.tile_pool(name="pvpool", bufs=3) as pvpool,
            tc.tile_pool(name="cpool", bufs=2) as cpool,
            tc.tile_pool(name="ypool", bufs=2) as ypool,
            tc.tile_pool(name="psum", bufs=8, space="PSUM") as psum,
        ):
            wt = wpool.tile([128, 23 * 128], bf)

            def W(i):
                return wt[:, i * 128 : (i + 1) * 128]

            def mm_into(p, contribs):
                # accumulate into an existing psum slice (one bank wide)
                n = len(contribs)
                for i, (wi, rhs) in enumerate(contribs):
                    nc.tensor.matmul(
                        p, W(wi), rhs, start=(i == 0), stop=(i == n - 1)
                    )

            def load(t):
                mbt = mbpool.tile([128, 4 * T], bf, tag="mb", name="mb_t")
                nc.sync.dma_start(
                    out=mbt[:, :].rearrange("p (c t) -> p c t", c=4),
                    in_=s4[t].unsqueeze(0).broadcast_to([128, 4, T]),
                )
                xt = xpool.tile([128, 10 * T], bf, tag="xt", name="x_t")
                # upper 6 chunks first: pvall (the big DVE op) needs only these
                nc.sync.dma_start(out=xt[:, 4 * T :], in_=x[t, :, 4 * T :])
                nc.sync.dma_start(out=xt[:, : 4 * T], in_=x[t, :, : 4 * T])
                return {"xt": xt, "mbt": mbt}

            def stage_g(st):
                # g = x0?' @ Wg (unscaled; only needs xt).  Emitted at the
                # head of each PE iteration so sgp is ready a full stage
                # before the t3 op that consumes it -- this breaks the
                # PE->Act->DVE chain that otherwise serializes the drain.
                xt = st["xt"]
                sgp = cpool.tile([128, 2 * T], bf, tag="sg", name="sg_t", bufs=4)
                for i, (wb, xg0) in enumerate(((12, 2), (17, 0))):
                    gp = psum.tile([128, T], f32, tag="psg", name="psg_t", bufs=2)
                    mm_into(
                        gp[:, :],
                        [
                            (wb + 0, xt[:, xg0 * T : (xg0 + 1) * T]),
                            (wb + 1, xt[:, (xg0 + 1) * T : (xg0 + 2) * T]),
                        ],
                    )
                    nc.scalar.copy(out=sgp[:, i * T : (i + 1) * T], in_=gp[:, :])
                st["sgp"] = sgp

            def stage_a(t, st):
                xt, mbt = st["xt"], st["mbt"]

                def mb(j, nch):
                    return (
                        mbt[:, j * T : (j + 1) * T]
                        .unsqueeze(1)
                        .broadcast_to([128, nch, T])
                    )

                # products: all 18 v-scaled chunks in ONE DVE op -- the
                # multiplier row advances by T per component c while the six
                # x1e/x1o chunks broadcast within each c
                pvall = pvpool.tile([128, 18 * T], bf, tag="pv", name="pv_t")
                nc.vector.tensor_mul(
                    pvall[:, :].rearrange("p (c k t) -> p c k t", c=3, k=6),
                    xt[:, 4 * T :]
                    .rearrange("p (k t) -> p k t", k=6)
                    .unsqueeze(1)
                    .broadcast_to([128, 3, 6, T]),
                    mbt[:, T:]
                    .rearrange("p (c t) -> p c t", c=3)
                    .unsqueeze(2)
                    .broadcast_to([128, 3, 6, T]),
                )

                def PV(c, ch):  # v_c-scaled chunk (ch is global 4..9)
                    o = c * 6 + (ch - 4)
                    return pvall[:, o * T : (o + 1) * T]

                def dpair(c):
                    # [128, 2, T]: {x1e_c*v_c (dot0o), x1o_c*v_c (dot0e)} --
                    # chunks c and 3+c within block c, a uniform 3T stride
                    o = c * 6 + c
                    return pvall[:, o * T : (o + 4) * T].rearrange(
                        "p (c t) -> p c t", c=4
                    )[:, 0::3, :]

                # dots for 0o|0e as a [128, 2T] pair (2 DVE ops, not 4)
                dta = cpool.tile([128, 2 * T], bf, tag="dta", name="dta_t", bufs=2)
                dotp = cpool.tile([128, 2 * T], bf, tag="dot", name="dot_t", bufs=2)
                dview = lambda ap: ap.rearrange("p (c t) -> p c t", c=2)
                nc.vector.tensor_add(dview(dta[:, :]), dpair(0), dpair(1))
                nc.vector.tensor_add(dview(dotp[:, :]), dview(dta[:, :]), dpair(2))
                dots = {0: dotp[:, T : 2 * T], 2: dotp[:, 0:T]}
                ps = pspool.tile([128, 10 * T], bf, tag="ps", name="ps_t")
                nc.vector.tensor_mul(
                    ps[:, :].rearrange("p (c t) -> p c t", c=10),
                    xt[:, :].rearrange("p (c t) -> p c t", c=10),
                    mb(0, 10),
                )

                def PS(ch):
                    return ps[:, ch * T : (ch + 1) * T]

                def XT(ch):
                    return xt[:, ch * T : (ch + 1) * T]

                yt = ypool.tile([128, 10 * T], bf, tag="yo", name="y_t")
                # 0e / 0o : both m-chunks in one [2T] psum, single Act copy
                # (the diag dot runs on DVE: a matmul here costs ~445ns at the
                # PE's sustained 1.2GHz clock, so un-collapsing loses)
                for base, wb, psa in ((0, 0, 0), (2, 6, 2)):
                    pp = psum.tile([128, 2 * T], f32, tag="ps0", name="ps0_t", bufs=2)
                    for m in range(2):
                        mm_into(
                            pp[:, m * T : (m + 1) * T],
                            [
                                (wb + 0 * 2 + m, PS(psa)),
                                (wb + 1 * 2 + m, PS(psa + 1)),
                                (wb + 2 * 2 + m, dots[base]),
                            ],
                        )
                    nc.scalar.copy(
                        out=yt[:, base * T : (base + 2) * T], in_=pp[:, :]
                    )
                    # stream this parity pair's output immediately
                    nc.sync.dma_start(
                        out=y[t, :, base * T : (base + 2) * T],
                        in_=yt[:, base * T : (base + 2) * T],
                    )
                st.update({"ps": ps, "pvall": pvall, "yt": yt})

            def stage_b_dve(st):
                # t3[i,c] = v_c * g_i for both parities in one DVE op
                mbt, sgp = st["mbt"], st["sgp"]
                t3p = cpool.tile([128, 6 * T], bf, tag="t3", name="t3_t", bufs=4)
                nc.vector.tensor_mul(
                    t3p[:, :].rearrange("p (i c t) -> p i c t", i=2, c=3),
                    mbt[:, T:]
                    .rearrange("p (c t) -> p c t", c=3)
                    .unsqueeze(1)
                    .broadcast_to([128, 2, 3, T]),
                    sgp[:, :]
                    .rearrange("p (i t) -> p i t", i=2)
                    .unsqueeze(2)
                    .broadcast_to([128, 2, 3, T]),
                )
                st["t3p"] = t3p

            def stage_b_pe(t, st):
                ps, pvall, yt, t3p = st["ps"], st["pvall"], st["yt"], st["t3p"]

                def PS(ch):
                    return ps[:, ch * T : (ch + 1) * T]

                def PV(c, ch):
                    o = c * 6 + (ch - 4)
                    return pvall[:, o * T : (o + 1) * T]

                for i, (wb, hb, cb, ob) in enumerate(
                    ((12, 4, 7, 4), (17, 7, 4, 7))
                ):
                    t3 = t3p[:, i * 3 * T : (i + 1) * 3 * T]

                    def contribs(c):
                        a, b = (c + 1) % 3, (c + 2) % 3
                        return [
                            (wb + 3, PV(b, cb + a)),      # k+: x1op_a * v_b
                            (wb + 4, PV(a, cb + b)),      # k-: x1op_b * v_a
                            (22, t3[:, c * T : (c + 1) * T]),  # += v_c * g
                            (wb + 2, PS(hb + c)),         # h: x1par_c * s (last:
                        ]                                 # ps lands latest)

                    # components 0,1 share a [2T] psum + one copy; c=2 alone
                    pp = psum.tile([128, 2 * T], f32, tag="ps1", name="ps1_t", bufs=1)
                    for c in range(2):
                        mm_into(pp[:, c * T : (c + 1) * T], contribs(c))
                    pc2 = psum.tile([128, T], f32, tag="psg", name="ps1c_t", bufs=2)
                    mm_into(pc2[:, :], contribs(2))
                    nc.scalar.copy(out=yt[:, ob * T : (ob + 2) * T], in_=pp[:, :])
                    nc.scalar.copy(
                        out=yt[:, (ob + 2) * T : (ob + 3) * T], in_=pc2[:, :]
                    )
                    # stream this parity's output while the other computes
                    nc.sync.dma_start(
                        out=y[t, :, ob * T : (ob + 3) * T],
                        in_=yt[:, ob * T : (ob + 3) * T],
                    )

            # software pipeline: loads prefetched one tile ahead, stage B
            # (combines + 1e/1o matmuls + store) one tile behind stage A
            states = {0: load(0)}
            # weights load queued after tile 0's data so the DVE-critical
            # descriptors go out first (PE touches weights later anyway)
            nc.sync.dma_start(out=wt[:, :], in_=w[:, :])
            for t in range(nt):
                if t + 1 < nt:
                    states[t + 1] = load(t + 1)
                stage_g(states[t])
                if t >= 1:
                    stage_b_dve(states[t - 1])
                    stage_b_pe(t - 1, states[t - 1])
                stage_a(t, states[t])
                if t >= 1:
                    del states[t - 1]
            stage_b_dve(states[nt - 1])
            stage_b_pe(nt - 1, states[nt - 1])
    nc.finalize()
    return nc


_PROG_CACHE = {}


def _get_program(Bs):
    if Bs not in _PROG_CACHE:
        _PROG_CACHE[Bs] = _build_program(Bs)
    return _PROG_CACHE[Bs]


def run(inputs, trace=False, **kw):
    in1 = np.asarray(inputs["in1"], np.float32)
    in2 = np.asarray(inputs["in2"], np.float32)
    B = in1.shape[0]
    assert B % (N_CORES * T) == 0, B
    Bs = B // N_CORES

    wpk = _pack_weights(
        np.asarray(inputs["W0e"], np.float32),
        np.asarray(inputs["W0o"], np.float32),
        np.asarray(inputs["W1e"], np.float32),
        np.asarray(inputs["W1o"], np.float32),
    )

    in_maps = []
    for i in range(N_CORES):
        ssl = slice(i * Bs, (i + 1) * Bs)
        xs, s4s = _prep_shard(in1[ssl], in2[ssl])
        in_maps.append({"x": xs, "s4": s4s, "w": wpk})

    nc = _get_program(Bs)
    res = run_bass_kernel_spmd(nc, in_maps, list(range(N_CORES)), trace=trace, **kw)

    out = np.empty((B, 1280), np.float32)
    for i in range(N_CORES):
        out[i * Bs : (i + 1) * Bs] = _post_shard(res.results[i]["y"])
    return out, res


def kernel(**inputs):
    out, _ = run(inputs, trace=False)
    return out
